# revision 33
# baseline (speedup 1.0000x reference)
"""BoxFuse (sparse_attention) Trainium2 Bass kernel — bf16 + cT restructuring.

Data-parallel over batch: 32 batches -> 8 NeuronCores x 4 batches.

Key structure (validated vs fp32 reference, sim rel-err ~5.7e-3):
  - All matmuls in bf16 (1 cyc/row, no power throttle, 512-wide streaming).
  - The big Q projection (604 MMAC/batch) is replaced algebraically:
      att[n,l] = sum_f x_hat_v[n,f] * c[l,f],  c[l,f] = sum_d Wq[f,d] k[l,d]
    c costs only ~105 MMAC/batch since k has <=100 rows. qb lands in the
    exp bias via kqb[l] = k . qb (masked rows keep -30).
  - All activation transposes (x_hat_v, x_hat_box) run on the DMA engines
    (dma_start_transpose, 16x128 XBAR tiles) - zero PE transposes, zero
    PSUM->SBUF staging copies for them.
  - Box tokens padded 100->112 per batch (XBAR needs rows % 16 == 0);
    garbage columns land in kT/cT slots that are never read.
  - V projection computed directly in natural [l, d] layout (box tokens
    as stationary operand).
  - v_bias folded into the vit residual on host; k bias applied during the
    kT PSUM->SBUF cast; outputs written bf16, cast to f32 on host.
"""

import os
import numpy as np

if os.environ.get("JAX_PLATFORMS", "").strip() == "cpu":
    os.environ.pop("JAX_PLATFORMS")

B, NTOK, L, LOW, HIGH = 32, 576, 100, 1024, 1536
NCORES = 8
BPC = B // NCORES            # batches per core
LN_EPS = 1e-5
MASK_NEG = -30.0
HT_HIGH = HIGH // 128        # 12 h-tiles for box features
HT_LOW = LOW // 128          # 8 f-tiles for vit features
DT = LOW // 128              # 8 d-tiles of projected features
LP = 112                     # box tokens padded to XBAR row granularity
LBP = BPC * LP               # 448 padded batch-concat box token dim

_CACHE = {}


def _build(reps=1):
    import concourse.bacc as bacc
    import concourse.tile as tile
    import concourse.mybir as mybir

    F32 = mybir.dt.float32
    BF16 = mybir.dt.bfloat16
    AF = mybir.ActivationFunctionType
    ALU = mybir.AluOpType

    nc = bacc.Bacc("TRN2", target_bir_lowering=False, debug=False)

    vit_d = nc.dram_tensor("vit", [BPC, NTOK, LOW], BF16, kind="ExternalInput").ap()
    box_d = nc.dram_tensor("box", [BPC, L, HIGH], BF16, kind="ExternalInput").ap()
    qwT_d = nc.dram_tensor("qwT", [128, DT, LOW], BF16, kind="ExternalInput").ap()
    kw_d = nc.dram_tensor("kw", [128, HT_HIGH, LOW], BF16, kind="ExternalInput").ap()
    vw_d = nc.dram_tensor("vw", [128, HT_HIGH, LOW], BF16, kind="ExternalInput").ap()
    qb_d = nc.dram_tensor("qb", [128, DT], BF16, kind="ExternalInput").ap()
    kb_d = nc.dram_tensor("kb", [128, DT], F32, kind="ExternalInput").ap()
    msc_d = nc.dram_tensor("msc", [L, BPC], F32, kind="ExternalInput").ap()
    mbs_d = nc.dram_tensor("mbs", [L, BPC], F32, kind="ExternalInput").ap()
    ones_d = nc.dram_tensor("ones", [128, 1], BF16, kind="ExternalInput").ap()
    out_d = nc.dram_tensor("out", [BPC, NTOK, LOW], BF16, kind="ExternalOutput").ap()

    NT = [(t * 128, min(128, NTOK - t * 128)) for t in range(5)]
    CNT = [(0, 512), (512, 64)]          # att/epilogue free-dim chunks over n
    VNT = [(0, 512), (512, 512)]         # v natural d chunks

    with tile.TileContext(nc) as tc:
      for _rep in range(reps):
        with (
            tc.tile_pool(name="consts", bufs=1) as consts,
            tc.tile_pool(name="persist", bufs=1) as persist,
            tc.tile_pool(name="small", bufs=int(os.environ.get("BF_SM", "6"))) as small,
            tc.tile_pool(name="pp_mm", bufs=int(os.environ.get("BF_MM", "2")), space="PSUM") as pp_mm,
            tc.tile_pool(name="pp_att", bufs=int(os.environ.get("BF_ATT", "6")), space="PSUM") as pp_att,
            tc.tile_pool(name="xTp", bufs=int(os.environ.get("BF_XT", "4"))) as xTp,
            tc.tile_pool(name="stageB", bufs=int(os.environ.get("BF_SB", "3"))) as stageB,
        ):
            ones = consts.tile([128, 1], BF16, tag="ones")
            nc.sync.dma_start(ones[:], ones_d)
            msc = consts.tile([128, BPC], F32, tag="msc")
            nc.sync.dma_start(msc[:L, :], msc_d)
            mbs = consts.tile([128, BPC], F32, tag="mbs")
            nc.sync.dma_start(mbs[:L, :], mbs_d)
            qb = consts.tile([128, DT], BF16, tag="qb")
            nc.sync.dma_start(qb[:], qb_d)
            kb = consts.tile([128, DT], F32, tag="kb")
            nc.sync.dma_start(kb[:], kb_d)
            eps_t = consts.tile([128, 1], F32, tag="eps")
            nc.vector.memset(eps_t[:], LN_EPS)

            kT = persist.tile([128, DT, LBP], BF16, tag="kT")    # k^T[d, l-pad]
            vnat = persist.tile([128, BPC, LOW], BF16, tag="v")  # v[l, d]
            cT = persist.tile([128, HT_LOW, LBP], BF16, tag="cT")  # c^T[f, l-pad]
            bias_all = persist.tile([128, BPC], F32, tag="biasall")
            # all vit tiles resident: prefetched during phase A, reread by
            # the epilogue residual add
            vit_all = persist.tile([128, BPC, 5, LOW], BF16, tag="vitall")
            qwT = persist.tile([128, DT, LOW], BF16, tag="qwT")  # Wq^T[d, f]

            def layernorm_stats(x_ap, rows, width):
                """x_ap [rows, width] bf16 -> (r, nmr) [rows, 1] f32."""
                chunks = width // 512
                st6 = small.tile([128, chunks, 6], F32, tag="st6")
                for c in range(chunks):
                    nc.vector.bn_stats(
                        st6[:rows, c, :], x_ap[:rows, c * 512:(c + 1) * 512]
                    )
                st2 = small.tile([128, 2], F32, tag="st2")
                nc.vector.bn_aggr(st2[:rows, :], st6[:rows, :, :])
                sd = small.tile([128, 1], F32, tag="sd")
                nc.scalar.activation(sd[:rows, :], st2[:rows, 1:2], AF.Sqrt,
                                     bias=eps_t[:rows, :], scale=1.0)
                r = small.tile([128, 1], F32, tag="r")
                nc.vector.reciprocal(r[:rows, :], sd[:rows, :])
                nmr = small.tile([128, 1], F32, tag="nmr")
                nc.vector.scalar_tensor_tensor(
                    nmr[:rows, :], st2[:rows, 0:1], -1.0, r[:rows, :],
                    op0=ALU.mult, op1=ALU.mult,
                )
                return r, nmr

            # ---------------- Phase A: box -> boxT16 -> kT, vnat ----------
            with (
                tc.tile_pool(name="wA", bufs=1) as wA,
                tc.tile_pool(name="stageA", bufs=int(os.environ.get("BF_SA", "2"))) as stageA,
                tc.tile_pool(name="boxTp", bufs=1) as boxTp,
            ):
                # box first on SP (it gates the whole phase-A PE pipeline),
                # then vit prefetches; weights go on the ACT queue as single
                # whole-tensor DMAs (few, large descriptors).
                bxs = []
                for b in range(BPC):
                    bx = stageA.tile([128, HIGH], BF16, tag="bx",
                                     name="bx", bufs=BPC)
                    nc.sync.dma_start(bx[:L, :], box_d[b])
                    bxs.append(bx)
                vw = wA.tile([128, HT_HIGH, LOW], BF16, tag="vw")
                nc.scalar.dma_start(vw[:], vw_d[:])
                kw = wA.tile([128, HT_HIGH, LOW], BF16, tag="kw")
                nc.scalar.dma_start(kw[:], kw_d[:])
                nc.scalar.dma_start(qwT[:], qwT_d[:])
                for b in range(BPC):
                    for t, (st, w) in enumerate(NT):
                        nc.sync.dma_start(vit_all[:w, b, t, :],
                                          vit_d[b, st:st + w, :])

                boxT16 = boxTp.tile([128, HT_HIGH, LBP], BF16, tag="boxT16")
                for b in range(BPC):
                    bx = bxs[b]
                    r, nmr = layernorm_stats(bx, L, HIGH)
                    xh = stageA.tile([128, HIGH], BF16, tag="xhb")
                    # rows L..LP are stale bits; their transposed columns are
                    # never read downstream (kT/cT slices stop at L).
                    nc.gpsimd.tensor_scalar(xh[:L, :], bx[:L, :], r[:L, :],
                                            nmr[:L, :], op0=ALU.mult, op1=ALU.add)
                    nc.sync.dma_start_transpose(
                        boxT16[:, :, b * LP:(b + 1) * LP], xh[:LP, :]
                    )
                    # V projection for this batch, natural [l, d] layout
                    for d0, dw in VNT:
                        ps = pp_mm.tile([128, 512], F32, tag="mm", name="ps")
                        for h in range(HT_HIGH):
                            nc.tensor.matmul(
                                ps[:L, :dw], boxT16[:, h, b * LP:b * LP + L],
                                vw[:, h, d0:d0 + dw],
                                start=(h == 0), stop=(h == HT_HIGH - 1),
                            )
                        nc.scalar.activation(vnat[:L, b, d0:d0 + dw], ps[:L, :dw],
                                             AF.Identity, scale=1.0)

                # x_hat_v transpose pipeline for ALL batches: vector/gpsimd/SP
                # flow during the K/cT PE work below (xTs consumed in phase B)
                xTs = []
                for b in range(BPC):
                    xT = xTp.tile([128, HT_LOW, NTOK], BF16, tag="xT", name="xT")
                    xTs.append(xT)
                    for t, (st, w) in enumerate(NT):
                        r, nmr = layernorm_stats(vit_all[:, b, t, :], w, LOW)
                        xh = stageB.tile([128, LOW], BF16, tag="xhv", name="xh")
                        nc.gpsimd.tensor_scalar(xh[:w, :], vit_all[:w, b, t, :],
                                                r[:w, :], nmr[:w, :],
                                                op0=ALU.mult, op1=ALU.add)
                        nc.sync.dma_start_transpose(xT[:, :, st:st + w], xh[:w, :])

                # K projection: kT[d, l-pad] = kw^T @ boxT16  (+ kb in cast)
                for dt in range(DT):
                    ps = pp_mm.tile([128, 512], F32, tag="mm")
                    for h in range(HT_HIGH):
                        nc.tensor.matmul(
                            ps[:, :LBP], kw[:, h, dt * 128:(dt + 1) * 128],
                            boxT16[:, h, :], start=(h == 0), stop=(h == HT_HIGH - 1),
                        )
                    nc.scalar.activation(kT[:, dt, :], ps[:, :LBP], AF.Identity,
                                         bias=kb[:, dt:dt + 1], scale=1.0)

                # cT[f, l-pad] = Wq^T @ kT, batch-concat (feeds phase-B att)
                for ft in range(HT_LOW):
                    ps = pp_mm.tile([128, 512], F32, tag="mm")
                    for dt in range(DT):
                        nc.tensor.matmul(
                            ps[:, :LBP], qwT[:, dt, ft * 128:(ft + 1) * 128],
                            kT[:, dt, :], start=(dt == 0), stop=(dt == DT - 1),
                        )
                    nc.scalar.activation(cT[:, ft, :], ps[:, :LBP],
                                         AF.Identity, scale=1.0)

                # kqb[l] = k . qb -> exp bias per batch (masked rows stay -30)
                for b in range(BPC):
                    psb = pp_att.tile([128, 512], F32, tag="att")
                    for dt in range(DT):
                        nc.tensor.matmul(psb[:L, :1], kT[:, dt, b * LP:b * LP + L],
                                         qb[:, dt:dt + 1],
                                         start=(dt == 0), stop=(dt == DT - 1))
                    nc.vector.scalar_tensor_tensor(
                        bias_all[:L, b:b + 1], psb[:L, :1], msc[:L, b:b + 1],
                        mbs[:L, b:b + 1], op0=ALU.mult, op1=ALU.add,
                    )

            # ---------------- Phase B: per batch ----------------
            with (
                tc.tile_pool(name="attp", bufs=int(os.environ.get("BF_ATTP", "2"))) as attp,
                tc.tile_pool(name="outp", bufs=int(os.environ.get("BF_OUT", "3"))) as outp,
            ):
                for b in range(BPC):
                    xT = xTs[b]
                    # attT[l, n] = cT . xT over f; exp with mask+bias fused
                    attT = attp.tile([128, NTOK], BF16, tag="attT")
                    for cs, cw in CNT:
                        ps = pp_att.tile([128, 512], F32, tag="att")
                        for ft in range(HT_LOW):
                            nc.tensor.matmul(
                                ps[:L, :cw], cT[:, ft, b * LP:b * LP + L],
                                xT[:, ft, cs:cs + cw],
                                start=(ft == 0), stop=(ft == HT_LOW - 1),
                            )
                        nc.scalar.activation(attT[:L, cs:cs + cw], ps[:L, :cw],
                                             AF.Exp, bias=bias_all[:L, b:b + 1],
                                             scale=msc[:L, b:b + 1])

                    # rowsum, reciprocal, att@v, epilogue
                    inv = small.tile([128, 5], F32, tag="inv")
                    for s, (st, w) in enumerate(NT):
                        pss = pp_att.tile([128, 512], F32, tag="att")
                        nc.tensor.matmul(pss[:w, :1], attT[:L, st:st + w],
                                         ones[:L, :], start=True, stop=True)
                        nc.vector.reciprocal(inv[:w, s:s + 1], pss[:w, :1])
                        outst = outp.tile([128, LOW], BF16, tag="outst")
                        for c in range(2):
                            cs = c * 512
                            psv = pp_att.tile([128, 512], F32, tag="att")
                            nc.tensor.matmul(
                                psv[:w, :], attT[:L, st:st + w],
                                vnat[:L, b, cs:cs + 512], start=True, stop=True,
                            )
                            if c == 0 or b == BPC - 1:
                                nc.vector.scalar_tensor_tensor(
                                    outst[:w, cs:cs + 512], psv[:w, :],
                                    inv[:w, s:s + 1], vit_all[:w, b, s, cs:cs + 512],
                                    op0=ALU.mult, op1=ALU.add,
                                )
                            else:
                                # scalar scales+casts, gpsimd adds the residual
                                nc.scalar.activation(
                                    outst[:w, cs:cs + 512], psv[:w, :],
                                    AF.Identity, scale=inv[:w, s:s + 1],
                                )
                                nc.gpsimd.tensor_tensor(
                                    outst[:w, cs:cs + 512], outst[:w, cs:cs + 512],
                                    vit_all[:w, b, s, cs:cs + 512], op=ALU.add,
                                )
                        nc.sync.dma_start(out_d[b, st:st + w, :], outst[:w, :])

    nc.compile()
    return nc


def kernel(**inputs):
    import ml_dtypes
    from concourse.bass_utils import run_bass_kernel_spmd

    BF16NP = ml_dtypes.bfloat16
    f32 = np.float32

    vit = np.asarray(inputs["vit_feat"], dtype=f32)
    box = np.asarray(inputs["box_feat"], dtype=f32)
    lengths = np.asarray(inputs["lengths"])

    def eff(ln_w, ln_b, w, bias, scale=1.0):
        w = np.asarray(w, f32)
        weff = (np.asarray(ln_w, f32)[:, None] * w) * f32(scale)
        beff = (np.asarray(ln_b, f32) @ w + np.asarray(bias, f32)) * f32(scale)
        return weff, beff

    att_scale = 1.0 / np.sqrt(np.float32(LOW))
    qw, qbv = eff(inputs["q_ln_w"], inputs["q_ln_b"], inputs["q_w"], inputs["q_b"],
                  att_scale)
    kw, kbv = eff(inputs["k_ln_w"], inputs["k_ln_b"], inputs["k_w"], inputs["k_b"])
    vw, vbv = eff(inputs["v_ln_w"], inputs["v_ln_b"], inputs["v_w"], inputs["v_b"])

    # layouts: qwT [p, dt, f] = Wq[f, dt*128+p]; kw/vw [p, h, d] = W[h*128+p, d]
    qwT16 = np.ascontiguousarray(
        qw.T.reshape(DT, 128, LOW).transpose(1, 0, 2)).astype(BF16NP)
    kw16 = np.ascontiguousarray(
        kw.reshape(HT_HIGH, 128, LOW).transpose(1, 0, 2)).astype(BF16NP)
    vw16 = np.ascontiguousarray(
        vw.reshape(HT_HIGH, 128, LOW).transpose(1, 0, 2)).astype(BF16NP)
    qbL = np.ascontiguousarray(qbv.reshape(DT, 128).T).astype(BF16NP)
    kbL = np.ascontiguousarray(kbv.reshape(DT, 128).T)

    vit16 = (vit + vbv[None, None, :]).astype(BF16NP)   # v bias folded here
    box16 = box.astype(BF16NP)

    valid = (np.arange(L)[None, :] < lengths[:, None].astype(np.int64))  # [B, L]
    msc_all = valid.astype(f32)
    mbs_all = np.where(valid, f32(0.0), f32(MASK_NEG))
    ones = np.ones((128, 1), dtype=BF16NP)

    if "nc" not in _CACHE:
        _CACHE["nc"] = _build()
    nc = _CACHE["nc"]

    in_maps = []
    for c in range(NCORES):
        sl = slice(c * BPC, (c + 1) * BPC)
        in_maps.append({
            "vit": np.ascontiguousarray(vit16[sl]),
            "box": np.ascontiguousarray(box16[sl]),
            "qwT": qwT16, "kw": kw16, "vw": vw16,
            "qb": qbL, "kb": kbL,
            "msc": np.ascontiguousarray(msc_all[sl].T),
            "mbs": np.ascontiguousarray(mbs_all[sl].T),
            "ones": ones,
        })

    _CACHE["in_maps"] = in_maps
    res = run_bass_kernel_spmd(nc, in_maps, core_ids=list(range(NCORES)))
    out = np.concatenate([np.asarray(res.results[c]["out"]) for c in range(NCORES)],
                         axis=0)
    return np.ascontiguousarray(out.astype(np.float32))


if __name__ == "__main__":
    rng = np.random.default_rng(0)
    ins = {
        "vit_feat": rng.standard_normal((B, NTOK, LOW)).astype(np.float32),
        "box_feat": rng.standard_normal((B, L, HIGH)).astype(np.float32),
        "lengths": rng.integers(0, L, (B,)).astype(np.int64),
        "q_ln_w": np.ones(LOW, np.float32), "q_ln_b": np.zeros(LOW, np.float32),
        "q_w": (rng.standard_normal((LOW, LOW)) * 0.02).astype(np.float32),
        "q_b": np.zeros(LOW, np.float32),
        "k_ln_w": np.ones(HIGH, np.float32), "k_ln_b": np.zeros(HIGH, np.float32),
        "k_w": (rng.standard_normal((HIGH, LOW)) * 0.02).astype(np.float32),
        "k_b": np.zeros(LOW, np.float32),
        "v_ln_w": np.ones(HIGH, np.float32), "v_ln_b": np.zeros(HIGH, np.float32),
        "v_w": (rng.standard_normal((HIGH, LOW)) * 0.02).astype(np.float32),
        "v_b": np.zeros(LOW, np.float32),
    }
    out = kernel(**ins)
    print("out", out.shape, out.dtype, np.abs(out).mean())


# revision 34
# speedup vs baseline: 1.1481x; 1.1481x over previous
"""BoxFuse (sparse_attention) Trainium2 Bass kernel — bf16 + cT restructuring.

Data-parallel over batch: 32 batches -> 8 NeuronCores x 4 batches.

Key structure (validated vs fp32 reference, sim rel-err ~5.7e-3):
  - All matmuls in bf16 (1 cyc/row, no power throttle, 512-wide streaming).
  - The big Q projection (604 MMAC/batch) is replaced algebraically:
      att[n,l] = sum_f x_hat_v[n,f] * c[l,f],  c[l,f] = sum_d Wq[f,d] k[l,d]
    c costs only ~105 MMAC/batch since k has <=100 rows. qb lands in the
    exp bias via kqb[l] = k . qb (masked rows keep -30).
  - All activation transposes (x_hat_v, x_hat_box) run on the DMA engines
    (dma_start_transpose, 16x128 XBAR tiles) - zero PE transposes, zero
    PSUM->SBUF staging copies for them.
  - Box tokens padded 100->112 per batch (XBAR needs rows % 16 == 0);
    garbage columns land in kT/cT slots that are never read.
  - V projection computed directly in natural [l, d] layout (box tokens
    as stationary operand).
  - v_bias folded into the vit residual on host; k bias applied during the
    kT PSUM->SBUF cast; outputs written bf16, cast to f32 on host.
"""

import os
import numpy as np

if os.environ.get("JAX_PLATFORMS", "").strip() == "cpu":
    os.environ.pop("JAX_PLATFORMS")

B, NTOK, L, LOW, HIGH = 32, 576, 100, 1024, 1536
NCORES = 8
BPC = B // NCORES            # batches per core
LN_EPS = 1e-5
MASK_NEG = -30.0
HT_HIGH = HIGH // 128        # 12 h-tiles for box features
HT_LOW = LOW // 128          # 8 f-tiles for vit features
DT = LOW // 128              # 8 d-tiles of projected features
LP = 112                     # box tokens padded to XBAR row granularity
LBP = BPC * LP               # 448 padded batch-concat box token dim

_CACHE = {}


def _build(reps=1):
    import concourse.bacc as bacc
    import concourse.tile as tile
    import concourse.mybir as mybir

    F32 = mybir.dt.float32
    BF16 = mybir.dt.bfloat16
    AF = mybir.ActivationFunctionType
    ALU = mybir.AluOpType

    nc = bacc.Bacc("TRN2", target_bir_lowering=False, debug=False)

    vit_d = nc.dram_tensor("vit", [BPC, NTOK, LOW], BF16, kind="ExternalInput").ap()
    box_d = nc.dram_tensor("box", [BPC, L, HIGH], BF16, kind="ExternalInput").ap()
    qwT_d = nc.dram_tensor("qwT", [128, DT, LOW], BF16, kind="ExternalInput").ap()
    kw_d = nc.dram_tensor("kw", [128, HT_HIGH, LOW], BF16, kind="ExternalInput").ap()
    vw_d = nc.dram_tensor("vw", [128, HT_HIGH, LOW], BF16, kind="ExternalInput").ap()
    qb_d = nc.dram_tensor("qb", [128, DT], BF16, kind="ExternalInput").ap()
    kb_d = nc.dram_tensor("kb", [128, DT], F32, kind="ExternalInput").ap()
    msc_d = nc.dram_tensor("msc", [L, BPC], F32, kind="ExternalInput").ap()
    mbs_d = nc.dram_tensor("mbs", [L, BPC], F32, kind="ExternalInput").ap()
    ones_d = nc.dram_tensor("ones", [128, 1], BF16, kind="ExternalInput").ap()
    out_d = nc.dram_tensor("out", [BPC, NTOK, LOW], BF16, kind="ExternalOutput").ap()

    NT = [(t * 128, min(128, NTOK - t * 128)) for t in range(5)]
    CNT = [(0, 512), (512, 64)]          # att/epilogue free-dim chunks over n
    VNT = [(0, 512), (512, 512)]         # v natural d chunks

    with tile.TileContext(nc) as tc:
      for _rep in range(reps):
        with (
            tc.tile_pool(name="consts", bufs=1) as consts,
            tc.tile_pool(name="persist", bufs=1) as persist,
            tc.tile_pool(name="small", bufs=int(os.environ.get("BF_SM", "6"))) as small,
            tc.tile_pool(name="pp_mm", bufs=int(os.environ.get("BF_MM", "2")), space="PSUM") as pp_mm,
            tc.tile_pool(name="pp_att", bufs=int(os.environ.get("BF_ATT", "6")), space="PSUM") as pp_att,
            tc.tile_pool(name="xTp", bufs=int(os.environ.get("BF_XT", "4"))) as xTp,
            tc.tile_pool(name="stageB", bufs=int(os.environ.get("BF_SB", "3"))) as stageB,
        ):
            ones = consts.tile([128, 1], BF16, tag="ones")
            nc.sync.dma_start(ones[:], ones_d)
            msc = consts.tile([128, BPC], F32, tag="msc")
            nc.sync.dma_start(msc[:L, :], msc_d)
            mbs = consts.tile([128, BPC], F32, tag="mbs")
            nc.sync.dma_start(mbs[:L, :], mbs_d)
            qb = consts.tile([128, DT], BF16, tag="qb")
            nc.sync.dma_start(qb[:], qb_d)
            kb = consts.tile([128, DT], F32, tag="kb")
            nc.sync.dma_start(kb[:], kb_d)
            eps_t = consts.tile([128, 1], F32, tag="eps")
            nc.vector.memset(eps_t[:], LN_EPS)

            kT = persist.tile([128, DT, LBP], BF16, tag="kT")    # k^T[d, l-pad]
            vnat = persist.tile([128, BPC, LOW], BF16, tag="v")  # v[l, d]
            cT = persist.tile([128, HT_LOW, LBP], BF16, tag="cT")  # c^T[f, l-pad]
            bias_all = persist.tile([128, BPC], F32, tag="biasall")
            # all vit tiles resident: prefetched during phase A, reread by
            # the epilogue residual add
            vit_all = persist.tile([128, BPC, 5, LOW], BF16, tag="vitall")
            qwT = persist.tile([128, DT, LOW], BF16, tag="qwT")  # Wq^T[d, f]

            def layernorm_stats(x_ap, rows, width):
                """x_ap [rows, width] bf16 -> (r, nmr) [rows, 1] f32."""
                chunks = width // 512
                st6 = small.tile([128, chunks, 6], F32, tag="st6")
                for c in range(chunks):
                    nc.vector.bn_stats(
                        st6[:rows, c, :], x_ap[:rows, c * 512:(c + 1) * 512]
                    )
                st2 = small.tile([128, 2], F32, tag="st2")
                nc.vector.bn_aggr(st2[:rows, :], st6[:rows, :, :])
                sd = small.tile([128, 1], F32, tag="sd")
                nc.scalar.activation(sd[:rows, :], st2[:rows, 1:2], AF.Sqrt,
                                     bias=eps_t[:rows, :], scale=1.0)
                r = small.tile([128, 1], F32, tag="r")
                nc.vector.reciprocal(r[:rows, :], sd[:rows, :])
                nmr = small.tile([128, 1], F32, tag="nmr")
                nc.vector.scalar_tensor_tensor(
                    nmr[:rows, :], st2[:rows, 0:1], -1.0, r[:rows, :],
                    op0=ALU.mult, op1=ALU.mult,
                )
                return r, nmr

            # ---------------- Phase A: box -> boxT16 -> kT, vnat ----------
            with (
                tc.tile_pool(name="wA", bufs=1) as wA,
                tc.tile_pool(name="stageA", bufs=int(os.environ.get("BF_SA", "2"))) as stageA,
                tc.tile_pool(name="boxTp", bufs=1) as boxTp,
            ):
                # box first on SP (it gates the whole phase-A PE pipeline),
                # then vit prefetches; weights go on the ACT queue as single
                # whole-tensor DMAs (few, large descriptors).
                bxs = []
                for b in range(BPC):
                    bx = stageA.tile([128, HIGH], BF16, tag="bx",
                                     name="bx", bufs=BPC)
                    nc.sync.dma_start(bx[:L, :], box_d[b])
                    bxs.append(bx)
                vw = wA.tile([128, HT_HIGH, LOW], BF16, tag="vw")
                nc.scalar.dma_start(vw[:], vw_d[:])
                kw = wA.tile([128, HT_HIGH, LOW], BF16, tag="kw")
                nc.scalar.dma_start(kw[:], kw_d[:])
                nc.scalar.dma_start(qwT[:], qwT_d[:])
                for b in range(BPC):
                    for t, (st, w) in enumerate(NT):
                        nc.sync.dma_start(vit_all[:w, b, t, :],
                                          vit_d[b, st:st + w, :])

                boxT16 = boxTp.tile([128, HT_HIGH, LBP], BF16, tag="boxT16")
                for b in range(BPC):
                    bx = bxs[b]
                    r, nmr = layernorm_stats(bx, L, HIGH)
                    xh = stageA.tile([128, HIGH], BF16, tag="xhb")
                    # rows L..LP are stale bits; their transposed columns are
                    # never read downstream (kT/cT slices stop at L).
                    nc.gpsimd.tensor_scalar(xh[:L, :], bx[:L, :], r[:L, :],
                                            nmr[:L, :], op0=ALU.mult, op1=ALU.add)
                    nc.sync.dma_start_transpose(
                        boxT16[:, :, b * LP:(b + 1) * LP], xh[:LP, :]
                    )
                    # V projection for this batch, natural [l, d] layout
                    for d0, dw in VNT:
                        ps = pp_mm.tile([128, 512], F32, tag="mm", name="ps")
                        for h in range(HT_HIGH):
                            nc.tensor.matmul(
                                ps[:L, :dw], boxT16[:, h, b * LP:b * LP + L],
                                vw[:, h, d0:d0 + dw],
                                start=(h == 0), stop=(h == HT_HIGH - 1),
                            )
                        nc.scalar.activation(vnat[:L, b, d0:d0 + dw], ps[:L, :dw],
                                             AF.Identity, scale=1.0)

                # x_hat_v transpose pipeline for ALL batches: vector/gpsimd/SP
                # flow during the K/cT PE work below (xTs consumed in phase B)
                xTs = []
                for b in range(BPC):
                    xT = xTp.tile([128, HT_LOW, NTOK], BF16, tag="xT", name="xT")
                    xTs.append(xT)
                    for t, (st, w) in enumerate(NT):
                        r, nmr = layernorm_stats(vit_all[:, b, t, :], w, LOW)
                        xh = stageB.tile([128, LOW], BF16, tag="xhv", name="xh")
                        nc.gpsimd.tensor_scalar(xh[:w, :], vit_all[:w, b, t, :],
                                                r[:w, :], nmr[:w, :],
                                                op0=ALU.mult, op1=ALU.add)
                        nc.sync.dma_start_transpose(xT[:, :, st:st + w], xh[:w, :])

                # K projection: kT[d, l-pad] = kw^T @ boxT16  (+ kb in cast)
                for dt in range(DT):
                    ps = pp_mm.tile([128, 512], F32, tag="mm")
                    for h in range(HT_HIGH):
                        nc.tensor.matmul(
                            ps[:, :LBP], kw[:, h, dt * 128:(dt + 1) * 128],
                            boxT16[:, h, :], start=(h == 0), stop=(h == HT_HIGH - 1),
                        )
                    nc.scalar.activation(kT[:, dt, :], ps[:, :LBP], AF.Identity,
                                         bias=kb[:, dt:dt + 1], scale=1.0)

                # cT[f, l-pad] = Wq^T @ kT, batch-concat (feeds phase-B att)
                for ft in range(HT_LOW):
                    ps = pp_mm.tile([128, 512], F32, tag="mm")
                    for dt in range(DT):
                        nc.tensor.matmul(
                            ps[:, :LBP], qwT[:, dt, ft * 128:(ft + 1) * 128],
                            kT[:, dt, :], start=(dt == 0), stop=(dt == DT - 1),
                        )
                    nc.scalar.activation(cT[:, ft, :], ps[:, :LBP],
                                         AF.Identity, scale=1.0)

                # kqb[l] = k . qb -> exp bias per batch (masked rows stay -30)
                for b in range(BPC):
                    psb = pp_att.tile([128, 512], F32, tag="att")
                    for dt in range(DT):
                        nc.tensor.matmul(psb[:L, :1], kT[:, dt, b * LP:b * LP + L],
                                         qb[:, dt:dt + 1],
                                         start=(dt == 0), stop=(dt == DT - 1))
                    nc.vector.scalar_tensor_tensor(
                        bias_all[:L, b:b + 1], psb[:L, :1], msc[:L, b:b + 1],
                        mbs[:L, b:b + 1], op0=ALU.mult, op1=ALU.add,
                    )

            # ---------------- Phase B: per batch ----------------
            with (
                tc.tile_pool(name="attp", bufs=int(os.environ.get("BF_ATTP", "2"))) as attp,
                tc.tile_pool(name="outp", bufs=int(os.environ.get("BF_OUT", "3"))) as outp,
            ):
                for b in range(BPC):
                    xT = xTs[b]
                    # attT[l, n] = cT . xT over f; exp with mask+bias fused
                    attT = attp.tile([128, NTOK], BF16, tag="attT")
                    for cs, cw in CNT:
                        ps = pp_att.tile([128, 512], F32, tag="att")
                        for ft in range(HT_LOW):
                            nc.tensor.matmul(
                                ps[:L, :cw], cT[:, ft, b * LP:b * LP + L],
                                xT[:, ft, cs:cs + cw],
                                start=(ft == 0), stop=(ft == HT_LOW - 1),
                            )
                        nc.scalar.activation(attT[:L, cs:cs + cw], ps[:L, :cw],
                                             AF.Exp, bias=bias_all[:L, b:b + 1],
                                             scale=msc[:L, b:b + 1])

                    # rowsum, reciprocal, att@v, epilogue
                    inv = small.tile([128, 5], F32, tag="inv")
                    for s, (st, w) in enumerate(NT):
                        pss = pp_att.tile([128, 512], F32, tag="att")
                        nc.tensor.matmul(pss[:w, :1], attT[:L, st:st + w],
                                         ones[:L, :], start=True, stop=True)
                        nc.vector.reciprocal(inv[:w, s:s + 1], pss[:w, :1])
                        outst = outp.tile([128, LOW], BF16, tag="outst")
                        for c in range(2):
                            cs = c * 512
                            psv = pp_att.tile([128, 512], F32, tag="att")
                            nc.tensor.matmul(
                                psv[:w, :], attT[:L, st:st + w],
                                vnat[:L, b, cs:cs + 512], start=True, stop=True,
                            )
                            if c == 0 or b == BPC - 1:
                                nc.vector.scalar_tensor_tensor(
                                    outst[:w, cs:cs + 512], psv[:w, :],
                                    inv[:w, s:s + 1], vit_all[:w, b, s, cs:cs + 512],
                                    op0=ALU.mult, op1=ALU.add,
                                )
                            else:
                                # scalar scales+casts, gpsimd adds the residual
                                nc.scalar.activation(
                                    outst[:w, cs:cs + 512], psv[:w, :],
                                    AF.Identity, scale=inv[:w, s:s + 1],
                                )
                                nc.gpsimd.tensor_tensor(
                                    outst[:w, cs:cs + 512], outst[:w, cs:cs + 512],
                                    vit_all[:w, b, s, cs:cs + 512], op=ALU.add,
                                )
                        nc.scalar.dma_start(out_d[b, st:st + w, :], outst[:w, :])

    nc.compile()
    return nc


def kernel(**inputs):
    import ml_dtypes
    from concourse.bass_utils import run_bass_kernel_spmd

    BF16NP = ml_dtypes.bfloat16
    f32 = np.float32

    vit = np.asarray(inputs["vit_feat"], dtype=f32)
    box = np.asarray(inputs["box_feat"], dtype=f32)
    lengths = np.asarray(inputs["lengths"])

    def eff(ln_w, ln_b, w, bias, scale=1.0):
        w = np.asarray(w, f32)
        weff = (np.asarray(ln_w, f32)[:, None] * w) * f32(scale)
        beff = (np.asarray(ln_b, f32) @ w + np.asarray(bias, f32)) * f32(scale)
        return weff, beff

    att_scale = 1.0 / np.sqrt(np.float32(LOW))
    qw, qbv = eff(inputs["q_ln_w"], inputs["q_ln_b"], inputs["q_w"], inputs["q_b"],
                  att_scale)
    kw, kbv = eff(inputs["k_ln_w"], inputs["k_ln_b"], inputs["k_w"], inputs["k_b"])
    vw, vbv = eff(inputs["v_ln_w"], inputs["v_ln_b"], inputs["v_w"], inputs["v_b"])

    # layouts: qwT [p, dt, f] = Wq[f, dt*128+p]; kw/vw [p, h, d] = W[h*128+p, d]
    qwT16 = np.ascontiguousarray(
        qw.T.reshape(DT, 128, LOW).transpose(1, 0, 2)).astype(BF16NP)
    kw16 = np.ascontiguousarray(
        kw.reshape(HT_HIGH, 128, LOW).transpose(1, 0, 2)).astype(BF16NP)
    vw16 = np.ascontiguousarray(
        vw.reshape(HT_HIGH, 128, LOW).transpose(1, 0, 2)).astype(BF16NP)
    qbL = np.ascontiguousarray(qbv.reshape(DT, 128).T).astype(BF16NP)
    kbL = np.ascontiguousarray(kbv.reshape(DT, 128).T)

    vit16 = (vit + vbv[None, None, :]).astype(BF16NP)   # v bias folded here
    box16 = box.astype(BF16NP)

    valid = (np.arange(L)[None, :] < lengths[:, None].astype(np.int64))  # [B, L]
    msc_all = valid.astype(f32)
    mbs_all = np.where(valid, f32(0.0), f32(MASK_NEG))
    ones = np.ones((128, 1), dtype=BF16NP)

    if "nc" not in _CACHE:
        _CACHE["nc"] = _build()
    nc = _CACHE["nc"]

    in_maps = []
    for c in range(NCORES):
        sl = slice(c * BPC, (c + 1) * BPC)
        in_maps.append({
            "vit": np.ascontiguousarray(vit16[sl]),
            "box": np.ascontiguousarray(box16[sl]),
            "qwT": qwT16, "kw": kw16, "vw": vw16,
            "qb": qbL, "kb": kbL,
            "msc": np.ascontiguousarray(msc_all[sl].T),
            "mbs": np.ascontiguousarray(mbs_all[sl].T),
            "ones": ones,
        })

    _CACHE["in_maps"] = in_maps
    res = run_bass_kernel_spmd(nc, in_maps, core_ids=list(range(NCORES)))
    out = np.concatenate([np.asarray(res.results[c]["out"]) for c in range(NCORES)],
                         axis=0)
    return np.ascontiguousarray(out.astype(np.float32))


if __name__ == "__main__":
    rng = np.random.default_rng(0)
    ins = {
        "vit_feat": rng.standard_normal((B, NTOK, LOW)).astype(np.float32),
        "box_feat": rng.standard_normal((B, L, HIGH)).astype(np.float32),
        "lengths": rng.integers(0, L, (B,)).astype(np.int64),
        "q_ln_w": np.ones(LOW, np.float32), "q_ln_b": np.zeros(LOW, np.float32),
        "q_w": (rng.standard_normal((LOW, LOW)) * 0.02).astype(np.float32),
        "q_b": np.zeros(LOW, np.float32),
        "k_ln_w": np.ones(HIGH, np.float32), "k_ln_b": np.zeros(HIGH, np.float32),
        "k_w": (rng.standard_normal((HIGH, LOW)) * 0.02).astype(np.float32),
        "k_b": np.zeros(LOW, np.float32),
        "v_ln_w": np.ones(HIGH, np.float32), "v_ln_b": np.zeros(HIGH, np.float32),
        "v_w": (rng.standard_normal((HIGH, LOW)) * 0.02).astype(np.float32),
        "v_b": np.zeros(LOW, np.float32),
    }
    out = kernel(**ins)
    print("out", out.shape, out.dtype, np.abs(out).mean())
